# revision 15
# baseline (speedup 1.0000x reference)
"""AtomAttentionDecoder — 8-core Bass/Tile kernel for TRN2.

Sharding: batch (4) x sequence-half (2) = 8 shards, one per NeuronCore.
Attention is local (128-key window); each shard computes an extended range
(owned 8192 atoms + 256-atom halo per side) so shards are independent.

Device kernel (per core, feature-major activations xT [C=128, atoms]):
  - token->atom gather via indirect DMA + PE transpose
  - 3 transformer blocks: LN (stats via ones-matmuls over partitions,
    broadcast via selector matmuls), QKV projections, windowed attention
    (scoresT = k^T q per (window, head); exp on ACT with the separable
    key-bias folded in as a per-partition bias; the query-side bias cancels
    in softmax; Z via staircase-selector matmuls; normalize fused with the
    PSUM->SBUF eviction), MLP with PSUM-accumulated second matmul.
  - final projection + PE transpose back to row-major, bf16 DMA out.

Host: a@W_a projection, pair-bias MLP (tiny), gather indices, masks,
weight folding (LN gains and 1/sqrt(dh) folded into the projections).
Falls back to the previous jax.jit path (and numpy) on any failure.
"""

import os
import numpy as np

B, N_TOK, N_ATOMS = 4, 2048, 16384
C_TOKEN, C_ATOM, C_PAIR = 384, 128, 16
N_Q, N_K, N_HEADS, N_BLOCKS = 32, 128, 4, 3
DH = C_ATOM // N_HEADS

HALO = 256
OWN = N_ATOMS // 2
N_EXT = OWN + 2 * HALO          # 8704
PAD = (N_K - N_Q) // 2          # 48
NW_EXT = N_EXT // N_Q           # 272
SCALE = float(1.0 / np.sqrt(DH))

NCH = 17                        # 512-col chunks over N_EXT
CH = 512
NPADC = N_EXT + 2 * PAD         # 8800
NG = 17                         # window groups of 16
GW = 16

_CACHE = {}
_DBG = bool(os.environ.get("KERNEL_DEBUG_TIMING"))


# ---------------------------------------------------------------------------
# Tile/axon compatibility patches (local walrus build allows only one
# sync-wait on SP ctrl instructions; NTFF profile hook not wired in antenv)
# ---------------------------------------------------------------------------

def _apply_patches():
    if _CACHE.get("patched"):
        return
    import concourse.tile as _tile
    from concourse.vector_clock import ScopedClock
    from concourse import mybir

    def _split_drain_and_barrier(self, tick_clock, wait_clock):
        nc = self.nc
        probe = nc.sync.nop(nofuse=True)
        wait_clock.add_sem_waits(probe.ins,
                                 ScopedClock({None: tick_clock.global_clock}))
        si = probe.ins.sync_info
        waits = list(si.on_wait) if si is not None and si.on_wait else []
        if len(waits) > 1:
            si.on_wait[:] = waits[:1]
            for w in waits[1:]:
                n2 = nc.sync.nop(nofuse=True)
                n2.ins.sync_info = mybir.SyncInfo(on_update=[], on_wait=[w])
        nc.sync.drain()
        nc.all_engine_barrier()
        assert self.sems is not None
        popped = nc._tile_sem_poison_stack.pop()
        assert popped is self._sem_poison
        nc.clear_and_free_semaphores(list(self.sems.allocated().values()))
        nc.all_engine_barrier()

    _tile.TileContext._drain_and_barrier = _split_drain_and_barrier

    # NTFF profile hook for trace=True under axon
    import sys, types
    if "antenv.axon_hooks" not in sys.modules:
        mod = types.ModuleType("antenv.axon_hooks")
        _hook = [None]
        mod.set_axon_ntff_profile_hook = lambda h: _hook.__setitem__(0, h)
        mod.get_axon_ntff_profile_hook = lambda: _hook[0]
        sys.modules["antenv.axon_hooks"] = mod
        try:
            import antenv
            antenv.axon_hooks = mod
            from trn_agent_boot.trn_boot import _ntff_profile_via_ctypes
            mod.set_axon_ntff_profile_hook(
                _ntff_profile_via_ctypes("/opt/axon/libaxon_pjrt.so"))
        except Exception:
            pass
    import concourse.bass_utils as bu
    bu.upload_artifacts = lambda tmpdir: f"local:{tmpdir}"
    _CACHE["patched"] = True


# ---------------------------------------------------------------------------
# Device kernel
# ---------------------------------------------------------------------------

def _build_nc():
    nc0 = _CACHE.get("nc")
    if nc0 is not None:
        return nc0
    _apply_patches()
    from contextlib import ExitStack
    import concourse.bass as bass
    import concourse.tile as tile
    from concourse import mybir

    bf = mybir.dt.bfloat16
    f32 = mybir.dt.float32

    nc = bass.Bass("TRN2")
    atok = nc.dram_tensor("atok", [N_TOK, C_ATOM], bf, kind="ExternalInput")
    idxs = nc.dram_tensor("idxs", [128, 68], mybir.dt.int32, kind="ExternalInput")
    biask = nc.dram_tensor("biask", [128, N_HEADS], f32, kind="ExternalInput")
    maskl = nc.dram_tensor("maskl", [128, CH], bf, kind="ExternalInput")
    maskr = nc.dram_tensor("maskr", [128, CH], bf, kind="ExternalInput")
    wq_d = nc.dram_tensor("wq", [N_BLOCKS, 128, 128], bf, kind="ExternalInput")
    wk_d = nc.dram_tensor("wk", [N_BLOCKS, 128, 128], bf, kind="ExternalInput")
    wv_d = nc.dram_tensor("wv", [N_BLOCKS, 128, 128], bf, kind="ExternalInput")
    wo_d = nc.dram_tensor("wo", [N_BLOCKS, 128, 128], bf, kind="ExternalInput")
    wt1_d = nc.dram_tensor("wt1", [N_BLOCKS, 128, 512], bf, kind="ExternalInput")
    wt2_d = nc.dram_tensor("wt2", [N_BLOCKS, 512, 128], bf, kind="ExternalInput")
    wout_d = nc.dram_tensor("wout", [128, 128], bf, kind="ExternalInput")
    selb_d = nc.dram_tensor("selb", [NCH, NCH * 128], bf, kind="ExternalInput")
    e17_d = nc.dram_tensor("e17", [128, 2 * NCH], bf, kind="ExternalInput")
    ces_d = nc.dram_tensor("ces", [128, 2 * N_HEADS], bf, kind="ExternalInput")
    exp4_d = nc.dram_tensor("exp4", [N_HEADS, 128], bf, kind="ExternalInput")
    iden_d = nc.dram_tensor("iden", [128, 128], bf, kind="ExternalInput")
    y_d = nc.dram_tensor("y", [OWN, C_ATOM], bf, kind="ExternalOutput")

    with tile.TileContext(nc) as tc, ExitStack() as ctx:
        state = ctx.enter_context(tc.tile_pool(name="state", bufs=1))
        work = ctx.enter_context(tc.tile_pool(name="work", bufs=2))
        stat = ctx.enter_context(tc.tile_pool(name="stat", bufs=1))
        expp = ctx.enter_context(tc.tile_pool(name="expp", bufs=8))
        psbig = ctx.enter_context(tc.tile_pool(name="psbig", bufs=2, space="PSUM"))
        pssc = ctx.enter_context(tc.tile_pool(name="pssc", bufs=2, space="PSUM"))
        pssm = ctx.enter_context(tc.tile_pool(name="pssm", bufs=2, space="PSUM"))
        psbc = ctx.enter_context(tc.tile_pool(name="psbc", bufs=2, space="PSUM"))

        # persistent activations (feature-major); hT and oT share one slot
        # (disjoint lifetimes: hT is consumed by the q/k/v projections, oT is
        # produced by attention afterwards)
        xT = state.tile([128, N_EXT], bf)
        kT = state.tile([128, NPADC], bf)
        qT = state.tile([128, N_EXT], bf)
        v4 = state.tile([128, 68 * 512], bf)   # (chunk j)(shift s)(C)

        # weights + constants
        wq_s = state.tile([128, N_BLOCKS * 128], bf)
        wk_s = state.tile([128, N_BLOCKS * 128], bf)
        wv_s = state.tile([128, N_BLOCKS * 128], bf)
        wo_s = state.tile([128, N_BLOCKS * 128], bf)
        wt1_s = state.tile([128, N_BLOCKS * 512], bf)
        wt2_s = state.tile([128, N_BLOCKS * 512], bf)
        wout_s = state.tile([128, 128], bf)
        selb_s = state.tile([NCH, NCH * 128], bf)
        e17_s = state.tile([128, 2 * NCH], bf)
        ces_s = state.tile([128, 2 * N_HEADS], bf)
        exp4_s = state.tile([N_HEADS, 128], bf)
        iden_s = state.tile([128, 128], bf)
        biask_s = state.tile([128, N_HEADS], f32)
        maskl_s = state.tile([128, CH], bf)
        maskr_s = state.tile([128, CH], bf)
        idx_s = state.tile([128, 68], mybir.dt.int32)
        epsr = state.tile([NCH, 1], f32)
        F = state.tile([NCH, 2 * CH], f32)      # [rstd | -mean*rstd]
        Fb = state.tile([NCH, 2 * CH], bf)      # bf16 copy for broadcasts

        dma = nc.sync.dma_start
        for l in range(N_BLOCKS):
            dma(out=wq_s[:, l * 128:(l + 1) * 128], in_=wq_d[l, :, :])
            dma(out=wk_s[:, l * 128:(l + 1) * 128], in_=wk_d[l, :, :])
            dma(out=wv_s[:, l * 128:(l + 1) * 128], in_=wv_d[l, :, :])
            dma(out=wo_s[:, l * 128:(l + 1) * 128], in_=wo_d[l, :, :])
            dma(out=wt1_s[:, l * 512:(l + 1) * 512], in_=wt1_d[l, :, :])
            for k in range(4):
                dma(out=wt2_s[:, l * 512 + k * 128:l * 512 + (k + 1) * 128],
                    in_=wt2_d[l, k * 128:(k + 1) * 128, :])
        dma(out=wout_s, in_=wout_d[:, :])
        dma(out=selb_s, in_=selb_d[:, :])
        dma(out=e17_s, in_=e17_d[:, :])
        dma(out=ces_s, in_=ces_d[:, :])
        dma(out=exp4_s, in_=exp4_d[:, :])
        dma(out=iden_s, in_=iden_d[:, :])
        dma(out=biask_s, in_=biask[:, :])
        dma(out=maskl_s, in_=maskl[:, :])
        dma(out=maskr_s, in_=maskr[:, :])
        dma(out=idx_s, in_=idxs[:, :])
        nc.vector.memset(epsr, 1e-5)
        nc.vector.memset(kT[:, 0:PAD], 0.0)
        nc.vector.memset(kT[:, PAD + N_EXT:], 0.0)

        # ------- input gather (token -> atom) + transpose to feature-major
        for j4 in range(17):
            pt = psbig.tile([128, 512], bf, tag="big")
            for s in range(4):
                j = j4 * 4 + s
                xr = work.tile([128, 128], bf, tag="xr")
                nc.gpsimd.indirect_dma_start(
                    out=xr[:, :], out_offset=None, in_=atok[:, :],
                    in_offset=bass.IndirectOffsetOnAxis(ap=idx_s[:, j:j + 1], axis=0))
                nc.tensor.transpose(out=pt[:, s * 128:(s + 1) * 128],
                                    in_=xr[:, :], identity=iden_s[:, :])
            nc.vector.tensor_copy(out=xT[:, j4 * 512:(j4 + 1) * 512], in_=pt[:, :])

        def layer_norm(src, padded):
            """LN over features (partitions) of src [128, N_EXT].
            Returns a fresh tile: [128, NPADC] zero-padded if padded else
            [128, N_EXT]."""
            hT = state.tile([128, NPADC], bf, tag="hslot")
            if padded:
                nc.vector.memset(hT[:, 0:PAD], 0.0)
                nc.vector.memset(hT[:, PAD + N_EXT:], 0.0)
            T1 = pssm.tile([NCH, CH], f32, tag="sm")
            T2 = pssm.tile([NCH, CH], f32, tag="sm")
            for ch in range(NCH):
                c0 = ch * CH
                nc.tensor.matmul(T1[:, :], e17_s[:, NCH - ch:2 * NCH - ch],
                                 src[:, c0:c0 + CH],
                                 start=(ch == 0), stop=(ch == NCH - 1),
                                 skip_group_check=True)
            for ch in range(NCH):
                c0 = ch * CH
                sq = work.tile([128, CH], bf, tag="sq")
                nc.vector.tensor_tensor(sq[:, :], src[:, c0:c0 + CH],
                                        src[:, c0:c0 + CH], mybir.AluOpType.mult)
                nc.tensor.matmul(T2[:, :], e17_s[:, NCH - ch:2 * NCH - ch],
                                 sq[:, :],
                                 start=(ch == 0), stop=(ch == NCH - 1),
                                 skip_group_check=True)
            m2 = stat.tile([NCH, CH], f32, tag="m2")
            nc.scalar.activation(out=m2[:, :], in_=T1[:, :],
                                 func=mybir.ActivationFunctionType.Square,
                                 scale=1.0 / 128.0)
            vv = stat.tile([NCH, CH], f32, tag="vv")
            nc.vector.tensor_scalar_mul(vv[:, :], T2[:, :], 1.0 / 128.0)
            nc.vector.tensor_tensor(vv[:, :], vv[:, :], m2[:, :],
                                    mybir.AluOpType.subtract)
            sr = stat.tile([NCH, CH], f32, tag="sr")
            nc.scalar.activation(out=sr[:, :], in_=vv[:, :],
                                 func=mybir.ActivationFunctionType.Sqrt,
                                 bias=epsr[:, :], scale=1.0)
            nc.vector.reciprocal(F[:, 0:CH], sr[:, :])
            mn = stat.tile([NCH, CH], f32, tag="mn")
            nc.vector.tensor_scalar_mul(mn[:, :], T1[:, :], -1.0 / 128.0)
            nc.vector.tensor_tensor(F[:, CH:2 * CH], mn[:, :], F[:, 0:CH],
                                    mybir.AluOpType.mult)
            nc.vector.tensor_copy(out=Fb[:, :], in_=F[:, :])
            for ch in range(NCH):
                c0 = ch * CH
                rb = psbc.tile([128, CH], f32, tag="bc")
                bb = psbc.tile([128, CH], f32, tag="bc")
                nc.tensor.matmul(rb[:, :], selb_s[:, ch * 128:(ch + 1) * 128],
                                 Fb[:, 0:CH], start=True, stop=True)
                nc.tensor.matmul(bb[:, :], selb_s[:, ch * 128:(ch + 1) * 128],
                                 Fb[:, CH:2 * CH], start=True, stop=True)
                tt = work.tile([128, CH], bf, tag="lnt")
                nc.vector.tensor_tensor(tt[:, :], src[:, c0:c0 + CH], rb[:, :],
                                        mybir.AluOpType.mult)
                nc.vector.tensor_tensor(hT[:, PAD + c0:PAD + c0 + CH],
                                        tt[:, :], bb[:, :], mybir.AluOpType.add)
            return hT

        for l in range(N_BLOCKS):
            lw = l * 128
            # ---- LN1
            hT = layer_norm(xT, True)

            # ---- q, k projections (feature-major)
            for ch in range(NCH):
                c0 = ch * CH
                pq = psbig.tile([128, CH], f32, tag="big")
                nc.tensor.matmul(pq[:, :], wq_s[:, lw:lw + 128],
                                 hT[:, PAD + c0:PAD + c0 + CH],
                                 start=True, stop=True)
                nc.vector.tensor_copy(out=qT[:, c0:c0 + CH], in_=pq[:, :])
                pk = psbig.tile([128, CH], f32, tag="big")
                nc.tensor.matmul(pk[:, :], wk_s[:, lw:lw + 128],
                                 hT[:, PAD + c0:PAD + c0 + CH],
                                 start=True, stop=True)
                nc.scalar.copy(out=kT[:, PAD + c0:PAD + c0 + CH], in_=pk[:, :])

            # ---- v in 4 shifted row-major copies
            for j in range(68):
                pv = psbig.tile([128, 512], f32, tag="big")
                for s in range(4):
                    nc.tensor.matmul(pv[:, s * 128:(s + 1) * 128],
                                     hT[:, j * 128 + s * 32:j * 128 + s * 32 + 128],
                                     wv_s[:, lw:lw + 128], start=True, stop=True)
                if j % 2 == 0:
                    nc.vector.tensor_copy(out=v4[:, j * 512:(j + 1) * 512],
                                          in_=pv[:, :])
                else:
                    nc.scalar.copy(out=v4[:, j * 512:(j + 1) * 512], in_=pv[:, :])

            # ---- windowed attention
            oT = state.tile([128, NPADC], bf, tag="hslot")
            for g in range(NG):
                exs = []
                z4 = pssm.tile([N_HEADS, CH], f32, tag="sm")
                for h in range(N_HEADS):
                    hp = h * DH
                    sc = pssc.tile([128, CH], f32, tag="sc")
                    for wl in range(GW):
                        w = g * GW + wl
                        nc.tensor.matmul(
                            sc[:, wl * 32:wl * 32 + 32],
                            kT[hp:hp + DH, w * 32:w * 32 + 128],
                            qT[hp:hp + DH, w * 32:w * 32 + 32],
                            start=True, stop=True, tile_position=(hp, 0))
                    ex = expp.tile([128, CH], bf, tag="ex")
                    nc.scalar.activation(out=ex[:, :], in_=sc[:, :],
                                         func=mybir.ActivationFunctionType.Exp,
                                         bias=biask_s[:, h:h + 1], scale=1.0)
                    if g == 0:
                        nc.vector.tensor_tensor(ex[:, :], ex[:, :], maskl_s[:, :],
                                                mybir.AluOpType.mult)
                    elif g == NG - 1:
                        nc.vector.tensor_tensor(ex[:, :], ex[:, :], maskr_s[:, :],
                                                mybir.AluOpType.mult)
                    nc.tensor.matmul(z4[:, :], ces_s[:, N_HEADS - h:2 * N_HEADS - h],
                                     ex[:, :], start=(h == 0), stop=(h == N_HEADS - 1),
                                     skip_group_check=True)
                    exs.append(ex)
                zi = stat.tile([N_HEADS, CH], f32, tag="zi")
                nc.vector.tensor_scalar_add(zi[:, :], z4[:, :], 1e-9)
                nc.vector.reciprocal(zi[:, :], zi[:, :])
                zib = stat.tile([N_HEADS, CH], bf, tag="zib")
                nc.vector.tensor_copy(out=zib[:, :], in_=zi[:, :])
                zx = psbc.tile([128, CH], f32, tag="bc")
                nc.tensor.matmul(zx[:, :], exp4_s[:, :], zib[:, :],
                                 start=True, stop=True)
                zxs = stat.tile([128, CH], f32, tag="zxs")
                nc.scalar.copy(out=zxs[:, :], in_=zx[:, :])
                ou = psbig.tile([128, CH], f32, tag="big")
                for h in range(N_HEADS):
                    hp = h * DH
                    ex = exs[h]
                    for wl in range(GW):
                        w = g * GW + wl
                        j, s = w // 4, w % 4
                        nc.tensor.matmul(
                            ou[hp:hp + DH, wl * 32:wl * 32 + 32],
                            v4[:, j * 512 + s * 128 + hp:j * 512 + s * 128 + hp + DH],
                            ex[:, wl * 32:wl * 32 + 32],
                            start=True, stop=True, tile_position=(0, hp))
                nc.vector.tensor_tensor(oT[:, g * CH:(g + 1) * CH], ou[:, :],
                                        zxs[:, :], mybir.AluOpType.mult)

            # ---- attention output projection + residual
            for ch in range(NCH):
                c0 = ch * CH
                pd = psbig.tile([128, CH], f32, tag="big")
                nc.tensor.matmul(pd[:, :], wo_s[:, lw:lw + 128],
                                 oT[:, c0:c0 + CH], start=True, stop=True)
                nc.vector.tensor_tensor(xT[:, c0:c0 + CH], xT[:, c0:c0 + CH],
                                        pd[:, :], mybir.AluOpType.add)

            # ---- LN2 + MLP
            hT = layer_norm(xT, False)
            for ch in range(NCH):
                c0 = ch * CH
                y2 = pssc.tile([128, CH], f32, tag="sc")
                for k in range(4):
                    uk = psbig.tile([128, CH], f32, tag="big")
                    nc.tensor.matmul(uk[:, :],
                                     wt1_s[:, l * 512 + k * 128:l * 512 + (k + 1) * 128],
                                     hT[:, PAD + c0:PAD + c0 + CH],
                                     start=True, stop=True)
                    ru = work.tile([128, CH], bf, tag="ru")
                    nc.scalar.activation(out=ru[:, :], in_=uk[:, :],
                                         func=mybir.ActivationFunctionType.Relu)
                    nc.tensor.matmul(y2[:, :],
                                     wt2_s[:, l * 512 + k * 128:l * 512 + (k + 1) * 128],
                                     ru[:, :], start=(k == 0), stop=(k == 3),
                                     skip_group_check=True)
                nc.vector.tensor_tensor(xT[:, c0:c0 + CH], xT[:, c0:c0 + CH],
                                        y2[:, :], mybir.AluOpType.add)

        # ---- output projection, transpose to row-major, DMA out (owned only)
        for ch in range(16):
            c0 = HALO + ch * CH
            yp = psbig.tile([128, CH], f32, tag="big")
            nc.tensor.matmul(yp[:, :], wout_s[:, :], xT[:, c0:c0 + CH],
                             start=True, stop=True)
            ys = work.tile([128, CH], bf, tag="ys")
            nc.vector.tensor_copy(out=ys[:, :], in_=yp[:, :])
            tp = pssc.tile([128, CH], bf, tag="sc")
            for s in range(4):
                nc.tensor.transpose(out=tp[:, s * 128:(s + 1) * 128],
                                    in_=ys[:, s * 128:(s + 1) * 128],
                                    identity=iden_s[:, :])
            yr = work.tile([128, CH], bf, tag="yr")
            nc.vector.tensor_copy(out=yr[:, :], in_=tp[:, :])
            nc.sync.dma_start(
                out=y_d[ch * 512:(ch + 1) * 512, :].rearrange(
                    "(s p) c -> p s c", p=128),
                in_=yr.rearrange("p (s c) -> p s c", s=4))

    _CACHE["nc"] = nc
    return nc


# ---------------------------------------------------------------------------
# Host-side preparation
# ---------------------------------------------------------------------------

def _host_consts():
    c = _CACHE.get("consts")
    if c is None:
        selb = np.zeros((NCH, NCH * 128), np.float32)
        for ch in range(NCH):
            selb[ch, ch * 128:(ch + 1) * 128] = 1.0
        e17 = np.zeros((128, 2 * NCH), np.float32)
        e17[:, NCH] = 1.0
        ces = np.zeros((128, 2 * N_HEADS), np.float32)
        ces[:, N_HEADS] = 1.0
        exp4 = np.zeros((N_HEADS, 128), np.float32)
        for h in range(N_HEADS):
            exp4[h, h * DH:(h + 1) * DH] = 1.0
        iden = np.eye(128, dtype=np.float32)
        c = (selb, e17, ces, exp4, iden)
        _CACHE["consts"] = c
    return c


def _edge_masks(core):
    gs = (core % 2) * OWN - HALO
    def mk(w0):
        m = np.ones((128, CH), np.float32)
        for wl in range(GW):
            w = w0 + wl
            kpos = gs + w * 32 - PAD + np.arange(128)
            bad = (kpos < 0) | (kpos >= N_ATOMS)
            if bad.any():
                m[bad, wl * 32:(wl + 1) * 32] = 0.0
        return m
    return mk(0), mk((NG - 1) * GW)


def _bass_kernel_path(a_tok, idx, biask_b, ws_bf):
    """Run the 8-core Bass kernel. Returns [8, OWN, 128] bf16 outputs."""
    import ml_dtypes
    bf16 = ml_dtypes.bfloat16
    from concourse.bass_utils import run_bass_kernel_spmd
    nc = _build_nc()
    selb, e17, ces, exp4, iden = _host_consts()
    wq, wk, wv, wo, wt1, wt2, wout = ws_bf

    in_maps = []
    for c in range(8):
        b, half = c // 2, c % 2
        gs = half * OWN - HALO
        ga = np.clip(np.arange(gs, gs + N_EXT), 0, N_ATOMS - 1)
        tok = idx[b][ga].astype(np.int32)
        idxs = np.ascontiguousarray(tok.reshape(68, 128).T)
        ml, mr = _edge_masks(c)
        in_maps.append({
            "atok": a_tok[b].astype(bf16),
            "idxs": idxs,
            "biask": np.ascontiguousarray(biask_b[b], np.float32),
            "maskl": ml.astype(bf16),
            "maskr": mr.astype(bf16),
            "wq": wq, "wk": wk, "wv": wv, "wo": wo,
            "wt1": wt1, "wt2": wt2, "wout": wout,
            "selb": selb.astype(bf16), "e17": e17.astype(bf16),
            "ces": ces.astype(bf16), "exp4": exp4.astype(np.float32),
            "iden": iden.astype(bf16),
        })
    _CACHE["last_in_maps"] = in_maps
    res = run_bass_kernel_spmd(nc, in_maps, list(range(8)))
    return [res.results[c]["y"] for c in range(8)]


def run_traced(core=0):
    """Re-run the bass kernel with NTFF tracing using the last call's inputs.
    Returns exec_time_ns (hardware, from neuron profile)."""
    from concourse.bass_utils import run_bass_kernel_spmd
    in_maps = _CACHE.get("last_in_maps")
    assert in_maps is not None, "call kernel() first"
    nc = _build_nc()
    res = run_bass_kernel_spmd(nc, in_maps, list(range(8)), trace=True)
    return res


# ---------------------------------------------------------------------------
# Fallback (previous jax.jit implementation)
# ---------------------------------------------------------------------------

def _masks_jax():
    m = _CACHE.get("masks")
    if m is None:
        war = np.arange(NW_EXT)[:, None] * N_Q - PAD + np.arange(N_K)
        m = np.empty((8, NW_EXT, N_K), np.float32)
        for c in range(8):
            gs = (c % 2) * OWN - HALO
            kpos = gs + war
            m[c] = np.where((kpos >= 0) & (kpos < N_ATOMS), 0.0, -1e9)
        _CACHE["masks"] = m
    return m


def _shard_math_dev(np_, jax, x, bias, mask, Wq, Wk, Wv, Wo, Wt1, Wt2, W_out):
    jnp = np_
    nw = NW_EXT

    def windows(t):
        blocks = t.reshape(nw + 3, N_Q, C_ATOM)
        return jnp.concatenate([blocks[j:j + nw] for j in range(4)], axis=1)

    def ln(h):
        m = h.mean(-1, keepdims=True)
        v = h.var(-1, keepdims=True)
        if jax is None:
            return (h - m) / np_.sqrt(v + 1e-5)
        return (h - m) * jax.lax.rsqrt(v + 1e-5)

    def pad_kv(t):
        return jnp.pad(t, ((PAD, PAD), (0, 0)))

    for l in range(N_BLOCKS):
        h = ln(x)
        q = (h @ Wq[l]).reshape(nw, N_Q, N_HEADS, DH)
        k = windows(pad_kv(h @ Wk[l])).reshape(nw, N_K, N_HEADS, DH)
        v = windows(pad_kv(h @ Wv[l])).reshape(nw, N_K, N_HEADS, DH)
        s = jnp.einsum('wqhd,wkhd->whqk', q, k) * SCALE
        s = s + bias[None] + mask[:, None, None, :]
        if jax is None:
            s = s - s.max(-1, keepdims=True)
            e = np_.exp(s)
            attn = e / e.sum(-1, keepdims=True)
        else:
            attn = jax.nn.softmax(s, axis=-1)
        o = jnp.einsum('whqk,wkhd->wqhd', attn, v).reshape(N_EXT, C_ATOM)
        x = x + o @ Wo[l]
        h2 = ln(x)
        relu = (lambda t: np_.maximum(t, 0.0)) if jax is None else jax.nn.relu
        x = x + relu(h2 @ Wt1[l]) @ Wt2[l]
        if jax is not None:
            x = jax.lax.optimization_barrier(x)
    return x @ W_out


def _get_fns(ws, bias_b, masks):
    fp = tuple(float(w.flat[i]) for w in ws for i in (0, w.size // 2, -1))
    fp += (bias_b.tobytes()[:256], masks[0].tobytes()[:256], "bf16io-v2")
    ent = _CACHE.get("ent")
    if ent is not None and ent[0] == fp:
        return ent[1], ent[2]
    try:
        import jax
        devs = jax.devices()
        if len(devs) < 8:
            raise RuntimeError("need 8 cores")
        import jax.numpy as jnp
        fns = []
        for c in range(8):
            def f(x, _b=bias_b[c // 2], _m=masks[c]):
                x = x.astype(jnp.float32)
                r = _shard_math_dev(jnp, jax, x, _b, _m, *ws)
                return r.astype(jnp.bfloat16)
            fns.append(jax.jit(f))
        fn = (fns, (devs, jax))
    except Exception:
        fn = (None, (None, None))
    _CACHE["ent"] = (fp, fn[0], fn[1])
    return fn


# ---------------------------------------------------------------------------
# Entry point
# ---------------------------------------------------------------------------

def kernel(a, r_l, atom_to_token_idx, W_a, W_out, W_cl, W_cm, W_mlp1, W_mlp2,
           W_pb, Wq, Wk, Wv, Wo, ln1_g, ln1_b, Wt1, Wt2, ln2_g, ln2_b):
    import time as _t
    a = np.asarray(a, np.float32)
    idx = np.asarray(atom_to_token_idx, np.int64)
    f32 = lambda w: np.asarray(w, np.float32)
    W_a, W_out, W_cl, W_cm, W_mlp1, W_mlp2, W_pb = map(
        f32, (W_a, W_out, W_cl, W_cm, W_mlp1, W_mlp2, W_pb))
    Wq, Wk, Wv, Wo, Wt1, Wt2 = map(f32, (Wq, Wk, Wv, Wo, Wt1, Wt2))
    ln1_g, ln1_b, ln2_g, ln2_b = map(f32, (ln1_g, ln1_b, ln2_g, ln2_b))

    # Fold LN gains into the following projections (exact for zero LN bias;
    # nonzero bias falls back to the general path).
    Wq_f = ln1_g[:, :, None] * Wq
    Wk_f = ln1_g[:, :, None] * Wk
    Wv_f = ln1_g[:, :, None] * Wv
    Wt1_f = ln2_g[:, :, None] * Wt1
    bq = np.einsum('lc,lcd->ld', ln1_b, Wq)
    bk = np.einsum('lc,lcd->ld', ln1_b, Wk)
    bv = np.einsum('lc,lcd->ld', ln1_b, Wv)
    bt1 = np.einsum('lc,lcd->ld', ln2_b, Wt1)
    has_bias = max(np.abs(x).max() for x in (bq, bk, bv, bt1)) > 0

    # Host: token projection (cheap) + pair bias
    a_tok = (a.reshape(-1, C_TOKEN) @ W_a).reshape(B, N_TOK, C_ATOM)
    ab = np.take_along_axis(a_tok, idx[:, :N_K, None], axis=1)
    p = ab @ (W_cl + W_cm)
    p = np.maximum(p, 0.0) @ W_mlp1
    p = np.maximum(p, 0.0) @ W_mlp2
    g = p @ W_pb                                  # [B, 128, H]
    bias_b = g[:, :N_Q, None, :] + g[:, None, :N_K, :]
    bias_b = np.ascontiguousarray(np.transpose(bias_b, (0, 3, 1, 2)))

    # --- primary: Bass kernel on 8 NeuronCores
    if not has_bias:
        try:
            import ml_dtypes
            bf16 = ml_dtypes.bfloat16
            Wq_s = (Wq_f * SCALE)
            ws_bf = tuple(np.ascontiguousarray(w, dtype=bf16) for w in
                          (Wq_s, Wk_f, Wv_f, Wo, Wt1_f, Wt2, W_out))
            ys = _bass_kernel_path(a_tok, idx, g, ws_bf)
            out = np.empty((B, N_ATOMS, C_ATOM), np.float32)
            for c in range(8):
                b, half = c // 2, c % 2
                out[b, half * OWN:(half + 1) * OWN] = ys[c]
            _CACHE["used_bass"] = True
            return out
        except Exception as e:
            if _DBG:
                import traceback
                traceback.print_exc()
            _CACHE.pop("nc", None)

    # --- fallback: jax.jit path (and numpy as last resort)
    masks = _masks_jax()

    def build_x(c, src, dtype):
        b, half = c // 2, c % 2
        x = np.zeros((N_EXT, C_ATOM), dtype)
        gs = half * OWN - HALO
        lo, hi = max(gs, 0), min(gs + N_EXT, N_ATOMS)
        x[lo - gs:hi - gs] = np.take(src[b], idx[b, lo:hi], axis=0)
        return x

    ws = (Wq_f, Wk_f, Wv_f, Wo, Wt1_f, Wt2, W_out)
    fns, (devs, jax) = (None, (None, None)) if has_bias else _get_fns(
        ws, bias_b, masks)
    if fns is not None:
        try:
            from concurrent.futures import ThreadPoolExecutor
            import ml_dtypes
            bf16 = ml_dtypes.bfloat16
            a_tok_bf = a_tok.astype(bf16)
            outs = np.empty((8, OWN, C_ATOM), np.float32)

            def run_core(c):
                x = build_x(c, a_tok_bf, bf16)
                xd = jax.device_put(x, devs[c])
                fut = fns[c](xd)
                outs[c] = np.asarray(fut)[HALO:HALO + OWN]

            with ThreadPoolExecutor(8) as ex:
                list(ex.map(run_core, range(8)))
            return outs.reshape(B, N_ATOMS, C_ATOM)
        except Exception:
            pass
    xs = np.stack([build_x(c, a_tok, np.float32) for c in range(8)])
    outs = np.stack([
        _np_shard_full(xs[c], bias_b[c // 2], masks[c], Wq_f, Wk_f, Wv_f,
                       Wo, Wt1_f, Wt2, W_out, bq, bk, bv, bt1)
        for c in range(8)])
    return outs.reshape(B, N_ATOMS, C_ATOM)


def _np_shard_full(x, bias, mask, Wq, Wk, Wv, Wo, Wt1, Wt2, W_out,
                   bq, bk, bv, bt1):
    key_idx = np.arange(NW_EXT)[:, None] * N_Q + np.arange(N_K)

    def ln(h):
        m = h.mean(-1, keepdims=True)
        v = h.var(-1, keepdims=True)
        return (h - m) / np.sqrt(v + 1e-5)

    for l in range(N_BLOCKS):
        h = ln(x)
        q = (h @ Wq[l] + bq[l]).reshape(NW_EXT, N_Q, N_HEADS, DH)
        kp = np.pad(h @ Wk[l] + bk[l], ((PAD, PAD), (0, 0)))
        vp = np.pad(h @ Wv[l] + bv[l], ((PAD, PAD), (0, 0)))
        k = kp[key_idx].reshape(NW_EXT, N_K, N_HEADS, DH)
        v = vp[key_idx].reshape(NW_EXT, N_K, N_HEADS, DH)
        s = np.einsum('wqhd,wkhd->whqk', q, k, optimize=True) * SCALE
        s = s + bias[None] + mask[:, None, None, :]
        s -= s.max(-1, keepdims=True)
        e = np.exp(s)
        attn = e / e.sum(-1, keepdims=True)
        o = np.einsum('whqk,wkhd->wqhd', attn, v, optimize=True)
        x = x + o.reshape(N_EXT, C_ATOM) @ Wo[l]
        h2 = ln(x)
        x = x + np.maximum(h2 @ Wt1[l] + bt1[l], 0.0) @ Wt2[l]
    return (x @ W_out)[HALO:HALO + OWN]


# revision 24
# speedup vs baseline: 2504.5354x; 2504.5354x over previous
"""AtomAttentionDecoder — 8-core Bass/Tile kernel for TRN2.

Sharding: batch (4) x sequence-half (2) = 8 shards, one per NeuronCore.
Attention is local (128-key window); each shard computes an extended range
(owned 8192 atoms + 256-atom halo per side) so shards are independent.

Device kernel (per core, feature-major activations xT [C=128, atoms]):
  - token->atom gather via indirect DMA + PE transpose
  - 3 transformer blocks: LN (stats via ones-matmuls over partitions,
    broadcast via selector matmuls), QKV projections, windowed attention
    (scoresT = k^T q per (window, head); exp on ACT with the separable
    key-bias folded in as a per-partition bias; the query-side bias cancels
    in softmax; Z via staircase-selector matmuls; normalize fused with the
    PSUM->SBUF eviction), MLP with PSUM-accumulated second matmul.
  - final projection + PE transpose back to row-major, bf16 DMA out.

Host: a@W_a projection, pair-bias MLP (tiny), gather indices, masks,
weight folding (LN gains and 1/sqrt(dh) folded into the projections).
Falls back to the previous jax.jit path (and numpy) on any failure.
"""

import os
import numpy as np

B, N_TOK, N_ATOMS = 4, 2048, 16384
C_TOKEN, C_ATOM, C_PAIR = 384, 128, 16
N_Q, N_K, N_HEADS, N_BLOCKS = 32, 128, 4, 3
DH = C_ATOM // N_HEADS

HALO = 256
OWN = N_ATOMS // 2
N_EXT = OWN + 2 * HALO          # 8704
PAD = (N_K - N_Q) // 2          # 48
NW_EXT = N_EXT // N_Q           # 272
SCALE = float(1.0 / np.sqrt(DH))

NCH = 17                        # 512-col chunks over N_EXT
CH = 512
NPADC = N_EXT + 2 * PAD         # 8800
NG = 17                         # window groups of 16
GW = 16

_CACHE = {}
_DBG = bool(os.environ.get("KERNEL_DEBUG_TIMING"))


# ---------------------------------------------------------------------------
# Tile/axon compatibility patches (local walrus build allows only one
# sync-wait on SP ctrl instructions; NTFF profile hook not wired in antenv)
# ---------------------------------------------------------------------------

def _apply_patches():
    if _CACHE.get("patched"):
        return
    import concourse.tile as _tile
    from concourse.vector_clock import ScopedClock
    from concourse import mybir

    def _split_drain_and_barrier(self, tick_clock, wait_clock):
        nc = self.nc
        probe = nc.sync.nop(nofuse=True)
        wait_clock.add_sem_waits(probe.ins,
                                 ScopedClock({None: tick_clock.global_clock}))
        si = probe.ins.sync_info
        waits = list(si.on_wait) if si is not None and si.on_wait else []
        if len(waits) > 1:
            si.on_wait[:] = waits[:1]
            for w in waits[1:]:
                n2 = nc.sync.nop(nofuse=True)
                n2.ins.sync_info = mybir.SyncInfo(on_update=[], on_wait=[w])
        nc.sync.drain()
        nc.all_engine_barrier()
        assert self.sems is not None
        popped = nc._tile_sem_poison_stack.pop()
        assert popped is self._sem_poison
        nc.clear_and_free_semaphores(list(self.sems.allocated().values()))
        nc.all_engine_barrier()

    _tile.TileContext._drain_and_barrier = _split_drain_and_barrier
    _CACHE["mybir"] = mybir

    # NTFF profile hook for trace=True under axon
    import sys, types
    if "antenv.axon_hooks" not in sys.modules:
        mod = types.ModuleType("antenv.axon_hooks")
        _hook = [None]
        mod.set_axon_ntff_profile_hook = lambda h: _hook.__setitem__(0, h)
        mod.get_axon_ntff_profile_hook = lambda: _hook[0]
        sys.modules["antenv.axon_hooks"] = mod
        try:
            import antenv
            antenv.axon_hooks = mod
            from trn_agent_boot.trn_boot import _ntff_profile_via_ctypes
            mod.set_axon_ntff_profile_hook(
                _ntff_profile_via_ctypes("/opt/axon/libaxon_pjrt.so"))
        except Exception:
            pass
    import concourse.bass_utils as bu
    bu.upload_artifacts = lambda tmpdir: f"local:{tmpdir}"
    _CACHE["patched"] = True


def _split_multi_waits(nc):
    """This walrus build allows only one sync-wait per instruction encoding;
    spread extra waits onto same-engine NoOps inserted just before."""
    mybir = _CACHE["mybir"]
    n = 0
    for _, bbh in nc.bb_map.items():
        bb = bbh.bb
        out = []
        for inst in bb.instructions:
            si = getattr(inst, "sync_info", None)
            waits = list(si.on_wait) if si is not None and si.on_wait else []
            if len(waits) > 1:
                si.on_wait[:] = [waits[-1]]
                for w in waits[:-1]:
                    nop = mybir.InstNoOp(name=f"{inst.name}-ws{n}", ins=[],
                                         outs=[])
                    n += 1
                    nop.engine = inst.engine
                    nop.sync_info = mybir.SyncInfo(on_update=[], on_wait=[w])
                    out.append(nop)
            out.append(inst)
        bb.instructions = out
    return n


# ---------------------------------------------------------------------------
# Device kernel
# ---------------------------------------------------------------------------

def _build_nc():
    nc0 = _CACHE.get("nc")
    if nc0 is not None:
        return nc0
    _apply_patches()
    from contextlib import ExitStack
    import concourse.bass as bass
    import concourse.tile as tile
    from concourse import mybir

    bf = mybir.dt.bfloat16
    f32 = mybir.dt.float32

    nc = bass.Bass("TRN2")
    atok = nc.dram_tensor("atok", [N_TOK, C_ATOM], bf, kind="ExternalInput")
    idxs = nc.dram_tensor("idxs", [128, 68], mybir.dt.int32, kind="ExternalInput")
    biask = nc.dram_tensor("biask", [128, N_HEADS], f32, kind="ExternalInput")
    maskl = nc.dram_tensor("maskl", [128, CH], bf, kind="ExternalInput")
    maskr = nc.dram_tensor("maskr", [128, CH], bf, kind="ExternalInput")
    wq_d = nc.dram_tensor("wq", [N_BLOCKS, 128, 128], bf, kind="ExternalInput")
    wk_d = nc.dram_tensor("wk", [N_BLOCKS, 128, 128], bf, kind="ExternalInput")
    wv_d = nc.dram_tensor("wv", [N_BLOCKS, 128, 128], bf, kind="ExternalInput")
    wo_d = nc.dram_tensor("wo", [N_BLOCKS, 128, 128], bf, kind="ExternalInput")
    wt1_d = nc.dram_tensor("wt1", [N_BLOCKS, 128, 512], bf, kind="ExternalInput")
    wt2_d = nc.dram_tensor("wt2", [N_BLOCKS, 512, 128], bf, kind="ExternalInput")
    wout_d = nc.dram_tensor("wout", [128, 128], bf, kind="ExternalInput")
    selb_d = nc.dram_tensor("selb", [NCH, NCH * 128], bf, kind="ExternalInput")
    e17_d = nc.dram_tensor("e17", [128, 2 * NCH], bf, kind="ExternalInput")
    ces_d = nc.dram_tensor("ces", [128, 2 * N_HEADS], bf, kind="ExternalInput")
    exp4_d = nc.dram_tensor("exp4", [32, 128], bf, kind="ExternalInput")
    iden_d = nc.dram_tensor("iden", [128, 128], bf, kind="ExternalInput")
    y_d = nc.dram_tensor("y", [OWN, C_ATOM], bf, kind="ExternalOutput")
    dbg = bool(os.environ.get("KERNEL_DEBUG_DUMP"))
    if dbg:
        dh_d = nc.dram_tensor("dh", [128, NPADC], bf, kind="ExternalOutput")
        dq_d = nc.dram_tensor("dq", [128, N_EXT], bf, kind="ExternalOutput")
        dk_d = nc.dram_tensor("dk", [128, NPADC], bf, kind="ExternalOutput")
        do_d = nc.dram_tensor("do", [128, N_EXT], bf, kind="ExternalOutput")
        dx_d = nc.dram_tensor("dx", [128, N_EXT], bf, kind="ExternalOutput")
        dv_d = nc.dram_tensor("dv", [128, 68 * 512], bf, kind="ExternalOutput")
        dex_d = nc.dram_tensor("dex", [128, 4 * CH], bf, kind="ExternalOutput")
        dzx_d = nc.dram_tensor("dzx", [128, CH], f32, kind="ExternalOutput")

    with tile.TileContext(nc) as tc, ExitStack() as ctx:
        state = ctx.enter_context(tc.tile_pool(name="state", bufs=1))
        work = ctx.enter_context(tc.tile_pool(name="work", bufs=2))
        stat = ctx.enter_context(tc.tile_pool(name="stat", bufs=1))
        expp = ctx.enter_context(tc.tile_pool(name="expp", bufs=8))
        psbig = ctx.enter_context(tc.tile_pool(name="psbig", bufs=2, space="PSUM"))
        pssc = ctx.enter_context(tc.tile_pool(name="pssc", bufs=2, space="PSUM"))
        pssm = ctx.enter_context(tc.tile_pool(name="pssm", bufs=2, space="PSUM"))
        psbc = ctx.enter_context(tc.tile_pool(name="psbc", bufs=2, space="PSUM"))

        # persistent activations (feature-major); hT and oT share one slot
        # (disjoint lifetimes: hT is consumed by the q/k/v projections, oT is
        # produced by attention afterwards)
        xT = state.tile([128, N_EXT], bf)
        kT = state.tile([128, NPADC], bf)
        qT = state.tile([128, N_EXT], bf)
        v4 = state.tile([128, 68 * 512], bf)   # (chunk j)(shift s)(C)

        # weights + constants
        wq_s = state.tile([128, N_BLOCKS * 128], bf)
        wk_s = state.tile([128, N_BLOCKS * 128], bf)
        wv_s = state.tile([128, N_BLOCKS * 128], bf)
        wo_s = state.tile([128, N_BLOCKS * 128], bf)
        wt1_s = state.tile([128, N_BLOCKS * 512], bf)
        wt2_s = state.tile([128, N_BLOCKS * 512], bf)
        wout_s = state.tile([128, 128], bf)
        selb_s = state.tile([NCH, NCH * 128], bf)
        e17_s = state.tile([128, 2 * NCH], bf)
        ces_s = state.tile([128, 2 * N_HEADS], bf)
        exp4_s = state.tile([32, 128], bf)
        zib32 = state.tile([32, CH], bf)
        iden_s = state.tile([128, 128], bf)
        biask_s = state.tile([128, N_HEADS], f32)
        maskl_s = state.tile([128, CH], bf)
        maskr_s = state.tile([128, CH], bf)
        idx_s = state.tile([128, 68], mybir.dt.int32)
        epsr = state.tile([NCH, 1], f32)
        F = state.tile([NCH, 2 * CH], f32)      # [rstd | -mean*rstd]
        Fb = state.tile([NCH, 2 * CH], bf)      # bf16 copy for broadcasts

        dma = nc.sync.dma_start
        for l in range(N_BLOCKS):
            dma(out=wq_s[:, l * 128:(l + 1) * 128], in_=wq_d[l, :, :])
            dma(out=wk_s[:, l * 128:(l + 1) * 128], in_=wk_d[l, :, :])
            dma(out=wv_s[:, l * 128:(l + 1) * 128], in_=wv_d[l, :, :])
            dma(out=wo_s[:, l * 128:(l + 1) * 128], in_=wo_d[l, :, :])
            dma(out=wt1_s[:, l * 512:(l + 1) * 512], in_=wt1_d[l, :, :])
            for k in range(4):
                dma(out=wt2_s[:, l * 512 + k * 128:l * 512 + (k + 1) * 128],
                    in_=wt2_d[l, k * 128:(k + 1) * 128, :])
        dma(out=wout_s, in_=wout_d[:, :])
        dma(out=selb_s, in_=selb_d[:, :])
        dma(out=e17_s, in_=e17_d[:, :])
        dma(out=ces_s, in_=ces_d[:, :])
        dma(out=exp4_s, in_=exp4_d[:, :])
        dma(out=iden_s, in_=iden_d[:, :])
        dma(out=biask_s, in_=biask[:, :])
        dma(out=maskl_s, in_=maskl[:, :])
        dma(out=maskr_s, in_=maskr[:, :])
        dma(out=idx_s, in_=idxs[:, :])
        nc.vector.memset(epsr, 1e-5)
        nc.vector.memset(zib32, 0.0)
        nc.vector.memset(kT[:, 0:PAD], 0.0)
        nc.vector.memset(kT[:, PAD + N_EXT:], 0.0)

        # ------- input gather (token -> atom) + transpose to feature-major
        for j4 in range(17):
            pt = psbig.tile([128, 512], bf, tag="big")
            for s in range(4):
                j = j4 * 4 + s
                xr = work.tile([128, 128], bf, tag="xr")
                nc.gpsimd.indirect_dma_start(
                    out=xr[:, :], out_offset=None, in_=atok[:, :],
                    in_offset=bass.IndirectOffsetOnAxis(ap=idx_s[:, j:j + 1], axis=0))
                nc.tensor.transpose(out=pt[:, s * 128:(s + 1) * 128],
                                    in_=xr[:, :], identity=iden_s[:, :])
            nc.vector.tensor_copy(out=xT[:, j4 * 512:(j4 + 1) * 512], in_=pt[:, :])

        def layer_norm(src, padded):
            """LN over features (partitions) of src [128, N_EXT].
            Returns a fresh tile: [128, NPADC] zero-padded if padded else
            [128, N_EXT]."""
            hT = state.tile([128, NPADC], bf, tag="hslot")
            if padded:
                nc.vector.memset(hT[:, 0:PAD], 0.0)
                nc.vector.memset(hT[:, PAD + N_EXT:], 0.0)
            T1 = pssm.tile([NCH, CH], f32, tag="sm")
            T2 = pssm.tile([NCH, CH], f32, tag="sm")
            for ch in range(NCH):
                c0 = ch * CH
                nc.tensor.matmul(T1[:, :], e17_s[:, NCH - ch:2 * NCH - ch],
                                 src[:, c0:c0 + CH],
                                 start=(ch == 0), stop=(ch == NCH - 1),
                                 skip_group_check=True)
            for ch in range(NCH):
                c0 = ch * CH
                sq = work.tile([128, CH], bf, tag="sq")
                nc.vector.tensor_tensor(sq[:, :], src[:, c0:c0 + CH],
                                        src[:, c0:c0 + CH], mybir.AluOpType.mult)
                nc.tensor.matmul(T2[:, :], e17_s[:, NCH - ch:2 * NCH - ch],
                                 sq[:, :],
                                 start=(ch == 0), stop=(ch == NCH - 1),
                                 skip_group_check=True)
            m2 = stat.tile([NCH, CH], f32, tag="m2")
            nc.scalar.activation(out=m2[:, :], in_=T1[:, :],
                                 func=mybir.ActivationFunctionType.Square,
                                 scale=1.0 / 128.0)
            vv = stat.tile([NCH, CH], f32, tag="vv")
            nc.vector.tensor_scalar_mul(vv[:, :], T2[:, :], 1.0 / 128.0)
            nc.vector.tensor_tensor(vv[:, :], vv[:, :], m2[:, :],
                                    mybir.AluOpType.subtract)
            sr = stat.tile([NCH, CH], f32, tag="sr")
            nc.scalar.activation(out=sr[:, :], in_=vv[:, :],
                                 func=mybir.ActivationFunctionType.Sqrt,
                                 bias=epsr[:, :], scale=1.0)
            nc.vector.reciprocal(F[:, 0:CH], sr[:, :])
            mn = stat.tile([NCH, CH], f32, tag="mn")
            nc.vector.tensor_scalar_mul(mn[:, :], T1[:, :], -1.0 / 128.0)
            nc.vector.tensor_tensor(F[:, CH:2 * CH], mn[:, :], F[:, 0:CH],
                                    mybir.AluOpType.mult)
            nc.vector.tensor_copy(out=Fb[:, :], in_=F[:, :])
            for ch in range(NCH):
                c0 = ch * CH
                rb = psbc.tile([128, CH], f32, tag="bc")
                bb = psbc.tile([128, CH], f32, tag="bc")
                nc.tensor.matmul(rb[:, :], selb_s[:, ch * 128:(ch + 1) * 128],
                                 Fb[:, 0:CH], start=True, stop=True)
                nc.tensor.matmul(bb[:, :], selb_s[:, ch * 128:(ch + 1) * 128],
                                 Fb[:, CH:2 * CH], start=True, stop=True)
                tt = work.tile([128, CH], bf, tag="lnt")
                nc.vector.tensor_tensor(tt[:, :], src[:, c0:c0 + CH], rb[:, :],
                                        mybir.AluOpType.mult)
                nc.vector.tensor_tensor(hT[:, PAD + c0:PAD + c0 + CH],
                                        tt[:, :], bb[:, :], mybir.AluOpType.add)
            return hT

        for l in range(N_BLOCKS):
            lw = l * 128
            # ---- LN1
            hT = layer_norm(xT, True)

            # ---- q, k projections (feature-major)
            for ch in range(NCH):
                c0 = ch * CH
                pq = psbig.tile([128, CH], f32, tag="big")
                nc.tensor.matmul(pq[:, :], wq_s[:, lw:lw + 128],
                                 hT[:, PAD + c0:PAD + c0 + CH],
                                 start=True, stop=True)
                nc.vector.tensor_copy(out=qT[:, c0:c0 + CH], in_=pq[:, :])
                pk = psbig.tile([128, CH], f32, tag="big")
                nc.tensor.matmul(pk[:, :], wk_s[:, lw:lw + 128],
                                 hT[:, PAD + c0:PAD + c0 + CH],
                                 start=True, stop=True)
                nc.scalar.copy(out=kT[:, PAD + c0:PAD + c0 + CH], in_=pk[:, :])

            if dbg and l == 0:
                nc.sync.dma_start(out=dh_d[:, :], in_=hT[:, :])
            # ---- v in 4 shifted row-major copies
            for j in range(68):
                pv = psbig.tile([128, 512], f32, tag="big")
                for s in range(4):
                    nc.tensor.matmul(pv[:, s * 128:(s + 1) * 128],
                                     hT[:, j * 128 + s * 32:j * 128 + s * 32 + 128],
                                     wv_s[:, lw:lw + 128], start=True, stop=True)
                if j % 2 == 0:
                    nc.vector.tensor_copy(out=v4[:, j * 512:(j + 1) * 512],
                                          in_=pv[:, :])
                else:
                    nc.scalar.copy(out=v4[:, j * 512:(j + 1) * 512], in_=pv[:, :])

            if dbg and l == 0:
                nc.sync.dma_start(out=dq_d[:, :], in_=qT[:, :])
                nc.sync.dma_start(out=dk_d[:, :], in_=kT[:, :])
                nc.sync.dma_start(out=dv_d[:, :], in_=v4[:, :])
            # ---- windowed attention
            oT = state.tile([128, NPADC], bf, tag="hslot")
            for g in range(NG):
                exs = []
                z4 = pssm.tile([N_HEADS, CH], f32, tag="sm")
                for h in range(N_HEADS):
                    hp = h * DH
                    sc = pssc.tile([128, CH], f32, tag="sc")
                    for wl in range(GW):
                        w = g * GW + wl
                        nc.tensor.matmul(
                            sc[:, wl * 32:wl * 32 + 32],
                            kT[hp:hp + DH, w * 32:w * 32 + 128],
                            qT[hp:hp + DH, w * 32:w * 32 + 32],
                            start=True, stop=True, tile_position=(hp, 0))
                    ex = expp.tile([128, CH], bf, tag="ex")
                    nc.scalar.activation(out=ex[:, :], in_=sc[:, :],
                                         func=mybir.ActivationFunctionType.Exp,
                                         bias=biask_s[:, h:h + 1], scale=1.0)
                    if g == 0:
                        nc.vector.tensor_tensor(ex[:, :], ex[:, :], maskl_s[:, :],
                                                mybir.AluOpType.mult)
                    elif g == NG - 1:
                        nc.vector.tensor_tensor(ex[:, :], ex[:, :], maskr_s[:, :],
                                                mybir.AluOpType.mult)
                    nc.tensor.matmul(z4[:, :], ces_s[:, N_HEADS - h:2 * N_HEADS - h],
                                     ex[:, :], start=(h == 0), stop=(h == N_HEADS - 1),
                                     skip_group_check=True)
                    if dbg and l == 0 and g == 6:
                        nc.sync.dma_start(out=dex_d[:, h * CH:(h + 1) * CH],
                                          in_=ex[:, :])
                    exs.append(ex)
                zi = stat.tile([N_HEADS, CH], f32, tag="zi")
                nc.vector.tensor_scalar_add(zi[:, :], z4[:, :], 1e-9)
                nc.vector.reciprocal(zi[:, :], zi[:, :])
                nc.vector.tensor_copy(out=zib32[0:N_HEADS, :], in_=zi[:, :])
                zx = psbc.tile([128, CH], f32, tag="bc")
                nc.tensor.matmul(zx[:, :], exp4_s[:, :], zib32[:, :],
                                 start=True, stop=True)
                zxs = stat.tile([128, CH], f32, tag="zxs")
                nc.scalar.copy(out=zxs[:, :], in_=zx[:, :])
                if dbg and l == 0 and g == 6:
                    nc.sync.dma_start(out=dzx_d[:, :], in_=zxs[:, :])
                ou = psbig.tile([128, CH], f32, tag="big")
                for h in range(N_HEADS):
                    hp = h * DH
                    ex = exs[h]
                    for wl in range(GW):
                        w = g * GW + wl
                        j, s = w // 4, w % 4
                        nc.tensor.matmul(
                            ou[hp:hp + DH, wl * 32:wl * 32 + 32],
                            v4[:, j * 512 + s * 128 + hp:j * 512 + s * 128 + hp + DH],
                            ex[:, wl * 32:wl * 32 + 32],
                            start=True, stop=True, tile_position=(0, hp))
                nc.vector.tensor_tensor(oT[:, g * CH:(g + 1) * CH], ou[:, :],
                                        zxs[:, :], mybir.AluOpType.mult)

            if dbg and l == 0:
                nc.sync.dma_start(out=do_d[:, :], in_=oT[:, :N_EXT])
            # ---- attention output projection + residual
            for ch in range(NCH):
                c0 = ch * CH
                pd = psbig.tile([128, CH], f32, tag="big")
                nc.tensor.matmul(pd[:, :], wo_s[:, lw:lw + 128],
                                 oT[:, c0:c0 + CH], start=True, stop=True)
                nc.vector.tensor_tensor(xT[:, c0:c0 + CH], xT[:, c0:c0 + CH],
                                        pd[:, :], mybir.AluOpType.add)

            if dbg and l == 0:
                nc.sync.dma_start(out=dx_d[:, :], in_=xT[:, :])
            # ---- LN2 + MLP
            hT = layer_norm(xT, False)
            for ch in range(NCH):
                c0 = ch * CH
                y2 = pssc.tile([128, CH], f32, tag="sc")
                for k in range(4):
                    uk = psbig.tile([128, CH], f32, tag="big")
                    nc.tensor.matmul(uk[:, :],
                                     wt1_s[:, l * 512 + k * 128:l * 512 + (k + 1) * 128],
                                     hT[:, PAD + c0:PAD + c0 + CH],
                                     start=True, stop=True)
                    ru = work.tile([128, CH], bf, tag="ru")
                    nc.scalar.activation(out=ru[:, :], in_=uk[:, :],
                                         func=mybir.ActivationFunctionType.Relu)
                    nc.tensor.matmul(y2[:, :],
                                     wt2_s[:, l * 512 + k * 128:l * 512 + (k + 1) * 128],
                                     ru[:, :], start=(k == 0), stop=(k == 3),
                                     skip_group_check=True)
                nc.vector.tensor_tensor(xT[:, c0:c0 + CH], xT[:, c0:c0 + CH],
                                        y2[:, :], mybir.AluOpType.add)

        # ---- output projection, transpose to row-major, DMA out (owned only)
        for ch in range(16):
            c0 = HALO + ch * CH
            yp = psbig.tile([128, CH], f32, tag="big")
            nc.tensor.matmul(yp[:, :], wout_s[:, :], xT[:, c0:c0 + CH],
                             start=True, stop=True)
            ys = work.tile([128, CH], bf, tag="ys")
            nc.vector.tensor_copy(out=ys[:, :], in_=yp[:, :])
            tp = pssc.tile([128, CH], bf, tag="sc")
            for s in range(4):
                nc.tensor.transpose(out=tp[:, s * 128:(s + 1) * 128],
                                    in_=ys[:, s * 128:(s + 1) * 128],
                                    identity=iden_s[:, :])
            yr = work.tile([128, CH], bf, tag="yr")
            nc.vector.tensor_copy(out=yr[:, :], in_=tp[:, :])
            nc.sync.dma_start(
                out=y_d[ch * 512:(ch + 1) * 512, :].rearrange(
                    "(s p) c -> p s c", p=128),
                in_=yr.rearrange("p (s c) -> p s c", s=4))

    _split_multi_waits(nc)
    _CACHE["nc"] = nc
    return nc


# ---------------------------------------------------------------------------
# Host-side preparation
# ---------------------------------------------------------------------------

def _host_consts():
    c = _CACHE.get("consts")
    if c is None:
        selb = np.zeros((NCH, NCH * 128), np.float32)
        for ch in range(NCH):
            selb[ch, ch * 128:(ch + 1) * 128] = 1.0
        e17 = np.zeros((128, 2 * NCH), np.float32)
        e17[:, NCH] = 1.0
        ces = np.zeros((128, 2 * N_HEADS), np.float32)
        ces[:, N_HEADS] = 1.0
        exp4 = np.zeros((32, 128), np.float32)
        for h in range(N_HEADS):
            exp4[h, h * DH:(h + 1) * DH] = 1.0
        iden = np.eye(128, dtype=np.float32)
        c = (selb, e17, ces, exp4, iden)
        _CACHE["consts"] = c
    return c


def _edge_masks(core):
    gs = (core % 2) * OWN - HALO
    def mk(w0):
        m = np.ones((128, CH), np.float32)
        for wl in range(GW):
            w = w0 + wl
            kpos = gs + w * 32 - PAD + np.arange(128)
            bad = (kpos < 0) | (kpos >= N_ATOMS)
            if bad.any():
                m[bad, wl * 32:(wl + 1) * 32] = 0.0
        return m
    return mk(0), mk((NG - 1) * GW)


def _bass_kernel_path(a_tok, idx, biask_b, ws_bf):
    """Run the 8-core Bass kernel. Returns [8, OWN, 128] bf16 outputs."""
    import ml_dtypes
    bf16 = ml_dtypes.bfloat16
    from concourse.bass_utils import run_bass_kernel_spmd
    nc = _build_nc()
    selb, e17, ces, exp4, iden = _host_consts()
    wq, wk, wv, wo, wt1, wt2, wout = ws_bf

    in_maps = []
    for c in range(8):
        b, half = c // 2, c % 2
        gs = half * OWN - HALO
        ga = np.clip(np.arange(gs, gs + N_EXT), 0, N_ATOMS - 1)
        tok = idx[b][ga].astype(np.int32)
        idxs = np.ascontiguousarray(tok.reshape(68, 128).T)
        ml, mr = _edge_masks(c)
        in_maps.append({
            "atok": a_tok[b].astype(bf16),
            "idxs": idxs,
            "biask": np.ascontiguousarray(biask_b[b], np.float32),
            "maskl": ml.astype(bf16),
            "maskr": mr.astype(bf16),
            "wq": wq, "wk": wk, "wv": wv, "wo": wo,
            "wt1": wt1, "wt2": wt2, "wout": wout,
            "selb": selb.astype(bf16), "e17": e17.astype(bf16),
            "ces": ces.astype(bf16), "exp4": exp4.astype(bf16),
            "iden": iden.astype(bf16),
        })
    _CACHE["last_in_maps"] = in_maps
    res = run_bass_kernel_spmd(nc, in_maps, list(range(8)))
    return [res.results[c]["y"] for c in range(8)]


def run_traced(core=0):
    """Re-run the bass kernel with NTFF tracing using the last call's inputs.
    Returns exec_time_ns (hardware, from neuron profile)."""
    from concourse.bass_utils import run_bass_kernel_spmd
    in_maps = _CACHE.get("last_in_maps")
    assert in_maps is not None, "call kernel() first"
    nc = _build_nc()
    res = run_bass_kernel_spmd(nc, in_maps, list(range(8)), trace=True)
    return res


# ---------------------------------------------------------------------------
# Fallback (previous jax.jit implementation)
# ---------------------------------------------------------------------------

def _masks_jax():
    m = _CACHE.get("masks")
    if m is None:
        war = np.arange(NW_EXT)[:, None] * N_Q - PAD + np.arange(N_K)
        m = np.empty((8, NW_EXT, N_K), np.float32)
        for c in range(8):
            gs = (c % 2) * OWN - HALO
            kpos = gs + war
            m[c] = np.where((kpos >= 0) & (kpos < N_ATOMS), 0.0, -1e9)
        _CACHE["masks"] = m
    return m


def _shard_math_dev(np_, jax, x, bias, mask, Wq, Wk, Wv, Wo, Wt1, Wt2, W_out):
    jnp = np_
    nw = NW_EXT

    def windows(t):
        blocks = t.reshape(nw + 3, N_Q, C_ATOM)
        return jnp.concatenate([blocks[j:j + nw] for j in range(4)], axis=1)

    def ln(h):
        m = h.mean(-1, keepdims=True)
        v = h.var(-1, keepdims=True)
        if jax is None:
            return (h - m) / np_.sqrt(v + 1e-5)
        return (h - m) * jax.lax.rsqrt(v + 1e-5)

    def pad_kv(t):
        return jnp.pad(t, ((PAD, PAD), (0, 0)))

    for l in range(N_BLOCKS):
        h = ln(x)
        q = (h @ Wq[l]).reshape(nw, N_Q, N_HEADS, DH)
        k = windows(pad_kv(h @ Wk[l])).reshape(nw, N_K, N_HEADS, DH)
        v = windows(pad_kv(h @ Wv[l])).reshape(nw, N_K, N_HEADS, DH)
        s = jnp.einsum('wqhd,wkhd->whqk', q, k) * SCALE
        s = s + bias[None] + mask[:, None, None, :]
        if jax is None:
            s = s - s.max(-1, keepdims=True)
            e = np_.exp(s)
            attn = e / e.sum(-1, keepdims=True)
        else:
            attn = jax.nn.softmax(s, axis=-1)
        o = jnp.einsum('whqk,wkhd->wqhd', attn, v).reshape(N_EXT, C_ATOM)
        x = x + o @ Wo[l]
        h2 = ln(x)
        relu = (lambda t: np_.maximum(t, 0.0)) if jax is None else jax.nn.relu
        x = x + relu(h2 @ Wt1[l]) @ Wt2[l]
        if jax is not None:
            x = jax.lax.optimization_barrier(x)
    return x @ W_out


def _get_fns(ws, bias_b, masks):
    fp = tuple(float(w.flat[i]) for w in ws for i in (0, w.size // 2, -1))
    fp += (bias_b.tobytes()[:256], masks[0].tobytes()[:256], "bf16io-v2")
    ent = _CACHE.get("ent")
    if ent is not None and ent[0] == fp:
        return ent[1], ent[2]
    try:
        import jax
        devs = jax.devices()
        if len(devs) < 8:
            raise RuntimeError("need 8 cores")
        import jax.numpy as jnp
        fns = []
        for c in range(8):
            def f(x, _b=bias_b[c // 2], _m=masks[c]):
                x = x.astype(jnp.float32)
                r = _shard_math_dev(jnp, jax, x, _b, _m, *ws)
                return r.astype(jnp.bfloat16)
            fns.append(jax.jit(f))
        fn = (fns, (devs, jax))
    except Exception:
        fn = (None, (None, None))
    _CACHE["ent"] = (fp, fn[0], fn[1])
    return fn


# ---------------------------------------------------------------------------
# Entry point
# ---------------------------------------------------------------------------

def kernel(a, r_l, atom_to_token_idx, W_a, W_out, W_cl, W_cm, W_mlp1, W_mlp2,
           W_pb, Wq, Wk, Wv, Wo, ln1_g, ln1_b, Wt1, Wt2, ln2_g, ln2_b):
    import time as _t
    a = np.asarray(a, np.float32)
    idx = np.asarray(atom_to_token_idx, np.int64)
    f32 = lambda w: np.asarray(w, np.float32)
    W_a, W_out, W_cl, W_cm, W_mlp1, W_mlp2, W_pb = map(
        f32, (W_a, W_out, W_cl, W_cm, W_mlp1, W_mlp2, W_pb))
    Wq, Wk, Wv, Wo, Wt1, Wt2 = map(f32, (Wq, Wk, Wv, Wo, Wt1, Wt2))
    ln1_g, ln1_b, ln2_g, ln2_b = map(f32, (ln1_g, ln1_b, ln2_g, ln2_b))

    # Fold LN gains into the following projections (exact for zero LN bias;
    # nonzero bias falls back to the general path).
    Wq_f = ln1_g[:, :, None] * Wq
    Wk_f = ln1_g[:, :, None] * Wk
    Wv_f = ln1_g[:, :, None] * Wv
    Wt1_f = ln2_g[:, :, None] * Wt1
    bq = np.einsum('lc,lcd->ld', ln1_b, Wq)
    bk = np.einsum('lc,lcd->ld', ln1_b, Wk)
    bv = np.einsum('lc,lcd->ld', ln1_b, Wv)
    bt1 = np.einsum('lc,lcd->ld', ln2_b, Wt1)
    has_bias = max(np.abs(x).max() for x in (bq, bk, bv, bt1)) > 0

    # Host: token projection (cheap) + pair bias
    a_tok = (a.reshape(-1, C_TOKEN) @ W_a).reshape(B, N_TOK, C_ATOM)
    ab = np.take_along_axis(a_tok, idx[:, :N_K, None], axis=1)
    p = ab @ (W_cl + W_cm)
    p = np.maximum(p, 0.0) @ W_mlp1
    p = np.maximum(p, 0.0) @ W_mlp2
    g = p @ W_pb                                  # [B, 128, H]
    bias_b = g[:, :N_Q, None, :] + g[:, None, :N_K, :]
    bias_b = np.ascontiguousarray(np.transpose(bias_b, (0, 3, 1, 2)))

    # --- primary: Bass kernel on 8 NeuronCores.
    # NOTE: numerics currently fail the 2e-2 gate (attention normalize stage,
    # ~4e-2) — enable via KERNEL_USE_BASS=1 while debugging. The jax path
    # below is the verified default.
    if not has_bias and os.environ.get("KERNEL_USE_BASS"):
        try:
            import ml_dtypes
            bf16 = ml_dtypes.bfloat16
            Wq_s = (Wq_f * SCALE)
            ws_bf = tuple(np.ascontiguousarray(w, dtype=bf16) for w in
                          (Wq_s, Wk_f, Wv_f, Wo, Wt1_f, Wt2, W_out))
            ys = _bass_kernel_path(a_tok, idx, g, ws_bf)
            out = np.empty((B, N_ATOMS, C_ATOM), np.float32)
            for c in range(8):
                b, half = c // 2, c % 2
                out[b, half * OWN:(half + 1) * OWN] = ys[c]
            _CACHE["used_bass"] = True
            return out
        except Exception as e:
            if _DBG:
                import traceback
                traceback.print_exc()
            _CACHE.pop("nc", None)

    # --- fallback: jax.jit path (and numpy as last resort)
    masks = _masks_jax()

    def build_x(c, src, dtype):
        b, half = c // 2, c % 2
        x = np.zeros((N_EXT, C_ATOM), dtype)
        gs = half * OWN - HALO
        lo, hi = max(gs, 0), min(gs + N_EXT, N_ATOMS)
        x[lo - gs:hi - gs] = np.take(src[b], idx[b, lo:hi], axis=0)
        return x

    ws = (Wq_f, Wk_f, Wv_f, Wo, Wt1_f, Wt2, W_out)
    fns, (devs, jax) = (None, (None, None)) if has_bias else _get_fns(
        ws, bias_b, masks)
    if fns is not None:
        try:
            from concurrent.futures import ThreadPoolExecutor
            import ml_dtypes
            bf16 = ml_dtypes.bfloat16
            a_tok_bf = a_tok.astype(bf16)
            outs = np.empty((8, OWN, C_ATOM), np.float32)

            def run_core(c):
                x = build_x(c, a_tok_bf, bf16)
                xd = jax.device_put(x, devs[c])
                fut = fns[c](xd)
                outs[c] = np.asarray(fut)[HALO:HALO + OWN]

            with ThreadPoolExecutor(8) as ex:
                list(ex.map(run_core, range(8)))
            return outs.reshape(B, N_ATOMS, C_ATOM)
        except Exception:
            pass
    xs = np.stack([build_x(c, a_tok, np.float32) for c in range(8)])
    outs = np.stack([
        _np_shard_full(xs[c], bias_b[c // 2], masks[c], Wq_f, Wk_f, Wv_f,
                       Wo, Wt1_f, Wt2, W_out, bq, bk, bv, bt1)
        for c in range(8)])
    return outs.reshape(B, N_ATOMS, C_ATOM)


def _np_shard_full(x, bias, mask, Wq, Wk, Wv, Wo, Wt1, Wt2, W_out,
                   bq, bk, bv, bt1):
    key_idx = np.arange(NW_EXT)[:, None] * N_Q + np.arange(N_K)

    def ln(h):
        m = h.mean(-1, keepdims=True)
        v = h.var(-1, keepdims=True)
        return (h - m) / np.sqrt(v + 1e-5)

    for l in range(N_BLOCKS):
        h = ln(x)
        q = (h @ Wq[l] + bq[l]).reshape(NW_EXT, N_Q, N_HEADS, DH)
        kp = np.pad(h @ Wk[l] + bk[l], ((PAD, PAD), (0, 0)))
        vp = np.pad(h @ Wv[l] + bv[l], ((PAD, PAD), (0, 0)))
        k = kp[key_idx].reshape(NW_EXT, N_K, N_HEADS, DH)
        v = vp[key_idx].reshape(NW_EXT, N_K, N_HEADS, DH)
        s = np.einsum('wqhd,wkhd->whqk', q, k, optimize=True) * SCALE
        s = s + bias[None] + mask[:, None, None, :]
        s -= s.max(-1, keepdims=True)
        e = np.exp(s)
        attn = e / e.sum(-1, keepdims=True)
        o = np.einsum('whqk,wkhd->wqhd', attn, v, optimize=True)
        x = x + o.reshape(N_EXT, C_ATOM) @ Wo[l]
        h2 = ln(x)
        x = x + np.maximum(h2 @ Wt1[l] + bt1[l], 0.0) @ Wt2[l]
    return (x @ W_out)[HALO:HALO + OWN]
